# revision 1
# baseline (speedup 1.0000x reference)
"""Causal multi-head self-attention on 8 TRN2 NeuronCores.

Sharding: core = (batch b, head-group g): 4 batches x 2 groups of 8 heads.
Host pre-transposes all operands so every TensorE matmul contracts over the
partition dim with zero on-device transposes:

  phase 1a: qk^T[n, i]  = sum_k Wqk[n, k] xT[k, i]      (lhsT=WqkT blk, rhs=xT)
  phase 1b: v[j, n]     = sum_k xT[k, j] WvT[k, n]      (lhsT=xT blk,   rhs=WvT)
  phase 2 (per head, per 512-wide i-chunk, per 128-deep j-block):
            S^T[j, i]   = sum_d kT[d, j] qT[d, i]       (lhsT=kT blk,   rhs=qT)
            A^T         = exp(S^T / 8) * causal_mask    (ACT + DVE)
            Yaug^T[n,i] = sum_j v_aug[j, n] A^T[j, i]   (lhsT=v_aug,    rhs=A^T)
              where v_aug has a ones column: row 64 of Yaug^T = softmax denom l
            y^T         = Yaug^T[0:64] * (1/l)          (recip + partition bcast)
  phase 3:  out[i, o]   = sum_n yT[n, i] WpT[n, o]      (lhsT=yT blk,   rhs=WpT)

All matmul operands are float32r (TF32-like, 1 cycle/row at N>=256, ~1.5e-4
matmul rel err); PSUM accumulation is fp32.  Softmax skips max-subtraction
(scores are O(+-10), exp is safe in fp32) so the denominator comes free from
the ones-column trick.  The two per-batch head-group partials are summed on
the host at gather time.
"""

import numpy as np

import concourse.mybir as mybir
import concourse.tile as tile
from concourse import bacc
from concourse.bass_utils import run_bass_kernel_spmd

F32 = mybir.dt.float32
F32R = mybir.dt.float32r
BF16 = mybir.dt.bfloat16
Exp = mybir.ActivationFunctionType.Exp

COMPUTE = "f32r"          # "f32r" | "bf16"


def _cdt():
    return F32R if COMPUTE == "f32r" else BF16


def set_compute(name):
    global COMPUTE
    assert name in ("f32r", "bf16")
    COMPUTE = name
    _CACHE.clear()

B, C, H = 4, 1024, 16
HPC = 8            # heads per core
HD = 64            # head dim
GQ = HPC * HD      # 512 columns per head group
P = 128
KB = C // P        # 8 k-blocks
SCALE = 0.125      # 1/sqrt(HD)


def build(T=2048, ps1_bufs=2, psS_bufs=2, psY_bufs=2, at_bufs=8, dup=1, ic=512, flat=True):
    CDT = _cdt()
    nT = T // P      # j-blocks
    nI = T // 512    # i-chunks
    nc = bacc.Bacc("TRN2", target_bir_lowering=False, debug=False, num_devices=8)

    xT = nc.dram_tensor("xT", [C, T], CDT, kind="ExternalInput").ap()
    wqkT = nc.dram_tensor("wqkT", [C, 2 * GQ], CDT, kind="ExternalInput").ap()
    wvT = nc.dram_tensor("wvT", [C, GQ], CDT, kind="ExternalInput").ap()
    wpT = nc.dram_tensor("wpT", [GQ, C], CDT, kind="ExternalInput").ap()
    maskT = nc.dram_tensor("maskT", [P, 2 * P], CDT, kind="ExternalInput").ap()
    onesT = nc.dram_tensor("onesT", [P, (T // P) * HPC], CDT, kind="ExternalInput").ap()
    out = nc.dram_tensor("out", [T, C], F32, kind="ExternalOutput").ap()

    from contextlib import ExitStack
    with tile.TileContext(nc) as tc:
      for _rep in range(dup):
        with tc.tile_pool(name="persist", bufs=1) as pe, ExitStack() as stk:
            pools = None
            if flat:
                pools = (
                    stk.enter_context(tc.tile_pool(name="psSf", bufs=psS_bufs, space="PSUM")),
                    stk.enter_context(tc.tile_pool(name="psYf", bufs=psY_bufs, space="PSUM")),
                    stk.enter_context(tc.tile_pool(name="ps3f", bufs=ps1_bufs, space="PSUM")),
                )

            qk_sb = pe.tile([P, 8 * T], CDT, tag="qk")      # n-blocks 0-3 q, 4-7 k
            v_sb = pe.tile([P, nT * HPC * (HD + 1)], CDT, tag="v")
            mask_sb = pe.tile([P, 2 * P], CDT, tag="mask")
            nc.sync.dma_start(mask_sb[:], maskT)
            nc.sync.dma_start(
                v_sb[:].rearrange("p (j h w) -> p j h w", j=nT, h=HPC)[:, :, :, HD:HD + 1],
                onesT.rearrange("p (j h) -> p j h", j=nT)[:, :, :, None])

            with tc.tile_pool(name="ph1", bufs=1) as p1, ExitStack() as stk1:
                ps1 = pools[2] if pools else stk1.enter_context(
                    tc.tile_pool(name="ps1", bufs=ps1_bufs, space="PSUM"))
                x_sb = p1.tile([P, KB * T], CDT, tag="x")
                wv_sb = p1.tile([P, KB * GQ], CDT, tag="wv")

                def qk_half(w_sb, half, mc_major):
                    order = ([(nb, mc) for mc in range(nI) for nb in range(4)]
                             if mc_major else
                             [(nb, mc) for nb in range(4) for mc in range(nI)])
                    for nb, mc in order:
                        pt = ps1.tile([P, 512], F32, tag="ps1")
                        for kb in range(KB):
                            nc.tensor.matmul(
                                pt[:],
                                w_sb[:, kb * GQ + nb * P: kb * GQ + (nb + 1) * P],
                                x_sb[:, kb * T + mc * 512: kb * T + (mc + 1) * 512],
                                start=(kb == 0), stop=(kb == KB - 1))
                        nc.vector.tensor_copy(
                            qk_sb[:, (4 * half + nb) * T + mc * 512:
                                  (4 * half + nb) * T + (mc + 1) * 512], pt[:])

                # ---- phase 1a-q first: wq + mc-chunked x DMAs pipeline the startup ----
                with tc.tile_pool(name="wqk0", bufs=1) as pw:
                    w_sb = pw.tile([P, KB * GQ], CDT, tag="w0")
                    for kb in range(KB):
                        nc.sync.dma_start(
                            w_sb[:, kb * GQ:(kb + 1) * GQ],
                            wqkT[kb * P:(kb + 1) * P, 0:GQ])
                    for mc in range(nI):
                        for kb in range(KB):
                            nc.sync.dma_start(
                                x_sb[:, kb * T + mc * 512: kb * T + (mc + 1) * 512],
                                xT[kb * P:(kb + 1) * P, mc * 512:(mc + 1) * 512])
                    qk_half(w_sb, 0, mc_major=True)

                # ---- phase 1b: v = x @ Wv^T (x now resident) ----
                for kb in range(KB):
                    nc.sync.dma_start(wv_sb[:, kb * GQ:(kb + 1) * GQ], wvT[kb * P:(kb + 1) * P, :])
                for jb in range(nT):
                    pt = ps1.tile([P, GQ], F32, tag="ps1")
                    for kb in range(KB):
                        nc.tensor.matmul(
                            pt[:],
                            x_sb[:, kb * T + jb * P: kb * T + (jb + 1) * P],
                            wv_sb[:, kb * GQ:(kb + 1) * GQ],
                            start=(kb == 0), stop=(kb == KB - 1))
                    vv = v_sb[:, jb * HPC * (HD + 1):(jb + 1) * HPC * (HD + 1)] \
                        .rearrange("p (h w) -> p h w", h=HPC)
                    nc.vector.tensor_copy(vv[:, :, 0:HD], pt[:].rearrange("p (h w) -> p h w", h=HPC))

                # ---- phase 1a-k ----
                with tc.tile_pool(name="wqk1", bufs=1) as pw:
                    w_sb = pw.tile([P, KB * GQ], CDT, tag="w1")
                    for kb in range(KB):
                        nc.sync.dma_start(
                            w_sb[:, kb * GQ:(kb + 1) * GQ],
                            wqkT[kb * P:(kb + 1) * P, GQ:2 * GQ])
                    qk_half(w_sb, 1, mc_major=False)

            # ---- phases 2+3 ----
            with tc.tile_pool(name="p23", bufs=1) as p23, \
                 tc.tile_pool(name="wrk", bufs=at_bufs) as wrk, \
                 tc.tile_pool(name="fin", bufs=2) as fin:
                yt_sb = p23.tile([P, 4 * T], CDT, tag="yt")
                wp_sb = p23.tile([P, 4 * C], CDT, tag="wp")
                phase23(nc, tc, T, nT, out, qk_sb, v_sb, mask_sb,
                        yt_sb, wp_sb, wpT, wrk, fin, ic, psS_bufs, psY_bufs, pools)
    return nc


def phase23(nc, tc, T, nT, out, qk_sb, v_sb, mask_sb, yt_sb, wp_sb, wpT,
            wrk, fin, ic, psS_bufs, psY_bufs, pools=None):
    """ci-outer flash attention with paired-exp full blocks + interleaved proj.

    Full (non-diagonal-crossing) j-blocks are processed in pairs sharing one
    [128, 1024] PSUM tile so a single ACT exp covers both (ACT instruction
    overhead is the phase-2 bottleneck).  After all heads finish an i-chunk,
    that chunk's 4 proj m-blocks run, spreading output DMA across phase 2.
    """
    CDT = _cdt()
    assert ic == 512
    nCh = T // ic      # i-chunks
    cpb = ic // P      # j-blocks per i-chunk span (4)
    for kb in range(4):
        nc.sync.dma_start(wp_sb[:, kb * C:(kb + 1) * C], wpT[kb * P:(kb + 1) * P, :])

    from contextlib import ExitStack
    with ExitStack() as stk:
        if pools:
            psS, psY, ps3 = pools
        else:
            psS = stk.enter_context(tc.tile_pool(name="psS", bufs=psS_bufs, space="PSUM"))
            psY = stk.enter_context(tc.tile_pool(name="psY", bufs=psY_bufs, space="PSUM"))
            ps3 = stk.enter_context(tc.tile_pool(name="ps3", bufs=2, space="PSUM"))

        for ci in range(nCh):
            jfull = cpb * ci               # full j-blocks (a=0), always even
            jmax = jfull + cpb
            for hp in range(0, HPC, 2):    # head PAIRS interleaved to hide exp latency
                hs = (hp, hp + 1)
                st = {}
                for h in hs:
                    st[h] = dict(
                        po=(h % 2) * HD, qc=(h // 2) * T, kc=(4 + h // 2) * T,
                        vc=h * (HD + 1),
                        py=psY.tile([HD + 1, 512], F32, tag="psY", name=f"py{ci}_{h}"))

                def st_mm(h, dst, jb, a):
                    s = st[h]
                    nc.tensor.matmul(
                        dst,
                        qk_sb[s["po"]:s["po"] + HD, s["kc"] + jb * P: s["kc"] + (jb + 1) * P],
                        qk_sb[s["po"]:s["po"] + HD,
                              s["qc"] + ci * 512 + a: s["qc"] + (ci + 1) * 512],
                        start=True, stop=True)

                def av_mm(h, jb, at_ap, a):
                    s = st[h]
                    nc.tensor.matmul(
                        s["py"][:, a:512],
                        v_sb[:, jb * HPC * (HD + 1) + s["vc"]:
                             jb * HPC * (HD + 1) + s["vc"] + HD + 1],
                        at_ap,
                        start=(jb == 0), stop=(jb == jmax - 1))

                for j0 in range(0, jfull, 2):          # paired full blocks, 2 heads zipped
                    ats = {}
                    for h in hs:
                        psp = psS.tile([P, 1024], F32, tag="psS", name=f"psp{ci}_{h}_{j0}")
                        st_mm(h, psp[:, 0:512], j0, 0)
                        st_mm(h, psp[:, 512:1024], j0 + 1, 0)
                        at = wrk.tile([P, 1024], CDT, tag="at", name=f"at{ci}_{h}_{j0}")
                        nc.scalar.activation(at[:], psp[:], Exp, scale=SCALE)
                        ats[h] = at
                    for h in hs:
                        av_mm(h, j0, ats[h][:, 0:512], 0)
                        av_mm(h, j0 + 1, ats[h][:, 512:1024], 0)

                for p_ in range(cpb):                  # crossing blocks: 2 heads packed
                    jb = jfull + p_
                    a = min(128 * p_, ic - 256)
                    w = 512 - a
                    mw = 128 * p_ - a + P              # 128, or 256 when clamped
                    psp = psS.tile([P, 1024], F32, tag="psS", name=f"psx{ci}_{hp}_{p_}")
                    st_mm(hs[0], psp[:, a:512], jb, a)
                    st_mm(hs[1], psp[:, 512:512 + w], jb, a)   # packed right after
                    at = wrk.tile([P, 1024], CDT, tag="at", name=f"atx{ci}_{hp}_{p_}")
                    nc.scalar.activation(at[:, a:512 + w], psp[:, a:512 + w], Exp, scale=SCALE)
                    nc.vector.tensor_mul(
                        at[:, a:a + mw], at[:, a:a + mw], mask_sb[:, 2 * P - mw:2 * P])
                    nc.vector.tensor_mul(
                        at[:, 512:512 + mw], at[:, 512:512 + mw], mask_sb[:, 2 * P - mw:2 * P])
                    av_mm(hs[0], jb, at[:, a:512], a)
                    av_mm(hs[1], jb, at[:, 512:512 + w], a)

                for h in hs:
                    s = st[h]
                    rt = fin.tile([1, 512], F32, tag="rt")
                    nc.vector.reciprocal(rt[:], s["py"][HD:HD + 1, :])
                    rb = fin.tile([HD, 512], F32, tag="rb")
                    nc.gpsimd.partition_broadcast(rb[:], rt[:])
                    nc.vector.tensor_mul(
                        yt_sb[s["po"]:s["po"] + HD,
                              s["qc"] + ci * 512: s["qc"] + (ci + 1) * 512],
                        s["py"][0:HD, :], rb[:])

            # ---- interleaved proj for this i-chunk's m-blocks ----
            for mb in range(cpb * ci, cpb * (ci + 1)):
                for oc in range(2):
                    po_ = ps3.tile([P, 512], F32, tag="ps1")
                    for nb in range(4):
                        nc.tensor.matmul(
                            po_[:],
                            yt_sb[:, nb * T + mb * P: nb * T + (mb + 1) * P],
                            wp_sb[:, nb * C + oc * 512: nb * C + (oc + 1) * 512],
                            start=(nb == 0), stop=(nb == 3))
                    ot = wrk.tile([P, 512], F32, tag="ot", bufs=3)
                    nc.vector.tensor_copy(ot[:], po_[:])
                    nc.sync.dma_start(out[mb * P:(mb + 1) * P, oc * 512:(oc + 1) * 512], ot[:])


_CACHE = {}


def get_nc(T=2048):
    if T not in _CACHE:
        nc = build(T)
        nc.compile()
        _CACHE[T] = nc
    return _CACHE[T]


def make_in_maps(x, W_attn, W_proj):
    Bx, T, Cx = x.shape
    Wq, Wk, Wv = W_attn[:Cx], W_attn[Cx:2 * Cx], W_attn[2 * Cx:]
    import ml_dtypes
    cv = (lambda a: np.ascontiguousarray(a)) if COMPUTE == "f32r" else (
        lambda a: np.ascontiguousarray(a).astype(ml_dtypes.bfloat16))
    r = np.arange(P)
    tri = (r[:, None] <= r[None, :]).astype(np.float32)
    mask = np.concatenate([np.zeros((P, P), np.float32), tri], axis=1)
    ones = np.ones((P, (T // P) * HPC), np.float32)
    in_maps = []
    for core in range(8):
        b, g = divmod(core, 2)
        rows = slice(g * GQ, (g + 1) * GQ)
        in_maps.append({
            "xT": cv(x[b].T),
            "wqkT": cv(np.concatenate([Wq[rows], Wk[rows]], 0).T),
            "wvT": cv(Wv[rows].T),
            "wpT": cv(W_proj[:, rows].T),
            "maskT": cv(mask),
            "onesT": cv(ones),
        })
    return in_maps


def kernel(x, W_attn, W_proj):
    x = np.asarray(x, dtype=np.float32)
    W_attn = np.asarray(W_attn, dtype=np.float32)
    W_proj = np.asarray(W_proj, dtype=np.float32)
    Bx, T, Cx = x.shape
    assert (Bx, Cx) == (B, C) and W_attn.shape == (3 * C, C) and W_proj.shape == (C, C)
    nc = get_nc(T)
    res = run_bass_kernel_spmd(nc, make_in_maps(x, W_attn, W_proj), list(range(8)))
    out = np.empty((Bx, T, Cx), np.float32)
    for b in range(Bx):
        out[b] = res.results[2 * b]["out"] + res.results[2 * b + 1]["out"]
    return out


if __name__ == "__main__":
    rng = np.random.default_rng(0)
    x = rng.standard_normal((B, 2048, C), dtype=np.float32)
    W_attn = rng.standard_normal((3 * C, C), dtype=np.float32) * (1.0 / np.sqrt(C))
    W_proj = rng.standard_normal((C, C), dtype=np.float32) * (1.0 / np.sqrt(C))
    out = kernel(x, W_attn, W_proj)
    print("out", out.shape, out.dtype, np.abs(out).max())

